# revision 27
# baseline (speedup 1.0000x reference)
"""Distributed Trainium2 kernel for nn_ContrastiveLoss (survival contrastive loss).

Strategy (8 NeuronCores, symmetric-triangle decomposition):
  host: quantile-bin rows into 4 risk groups; lay rows out so column-tile t
        (512 rows) holds group t%4 (interleaved) -> "same group" == tile
        distance d % 4 == 0; normalize in f32 and quantize to fp8e4 (2e-2
        tolerance; measured end-to-end rel err ~1e-5).
        Each unordered tile-pair {r, c} of the 16x16 grid is computed exactly
        once: core k owns strips k (col offsets d=0..8) and k+8 (d=8..15,
        mod 16).  A fixed local permutation pi makes the per-core program
        identical (SPMD): local col j <-> global tile (k + pi[j]) % 16, with
        pi placing strip-A's same-group cols (d=4,8) right after its diagonal.
  device (per core): fp8 DoubleRow matmuls (K=512 as 2 plane-pairs, 0.5
        cycles/row) -> sim row-blocks in PSUM; one exp per <=1536-wide
        group-pure batch on ACT (scale=10, bias=-10) with accum_out giving
        masked row sums; bf16 exp tiles accumulate per row-subtile; ones-
        matmul chains give per-column sums (the transposed tiles' row sums);
        diagonal extracted via dmask on DVE.
  host: route row/col partial sums + diagonals into den/pos per row,
        loss = mean(log(den) - log(pos)).
"""
import sys

sys.path.insert(0, "/opt/trn_rl_repo")
import numpy as np
import ml_dtypes

N, D, G, NCORES = 8192, 512, 4, 8
TEMP = 0.1
CT = 512                  # col tile (rows per tile)
NT = N // CT              # 16 col tiles
PI = [0, 4, 8, 1, 2, 3, 5, 6, 7, 12, 9, 10, 11, 13, 14, 15]
# local col j of core k <-> global tile (k + PI[j]) % 16
# strip A rows = local tile 0 (d=0); strip B rows = local tile 2 (d=8)
LB = 2

_built = None


def _build():
    from concourse import bacc, tile, mybir

    nc = bacc.Bacc(None, target_bir_lowering=False)
    f32 = mybir.dt.float32
    bf16 = mybir.dt.bfloat16
    f8 = mybir.dt.float8e4
    AF = mybir.ActivationFunctionType
    AX = mybir.AxisListType
    DR = mybir.MatmulPerfMode.DoubleRow

    z8t = nc.dram_tensor("z8t", [2, 128, 2, N], f8, kind="ExternalInput")
    dmask = nc.dram_tensor("dmask", [128, 4 * CT], bf16, kind="ExternalInput")
    ones = nc.dram_tensor("ones", [128, 16], bf16, kind="ExternalInput")
    rs = nc.dram_tensor("rs", [128, 40], f32, kind="ExternalOutput")
    cs = nc.dram_tensor("cs", [1, 12 * CT], f32, kind="ExternalOutput")
    # cols 13..15 and col 9 ship as raw bf16 exp tiles (summed on host):
    # their ones-chains would otherwise serialize after the final exps
    aend = nc.dram_tensor("aend", [4, 128, 4 * CT], bf16,
                          kind="ExternalOutput")

    with tile.TileContext(nc) as tc:
        with tc.tile_pool(name="cst", bufs=1) as cst, \
             tc.tile_pool(name="zt", bufs=1) as ztp, \
             tc.tile_pool(name="acc", bufs=1) as accp, \
             tc.tile_pool(name="sc", bufs=2) as scp, \
             tc.tile_pool(name="pb", bufs=2, space="PSUM") as psim, \
             tc.tile_pool(name="ps", bufs=2, space="PSUM") as pone:

            nc.scalar.add_instruction(
                mybir.InstLoadActFuncSet(
                    name=nc.get_next_instruction_name(),
                    act_func_set_id=6, ins=[], outs=[]))

            dmt = cst.tile([128, 4 * CT], bf16)
            onest = cst.tile([128, 16], bf16)
            bias10 = cst.tile([128, 1], f32)
            nc.vector.memset(bias10[:], -10.0)
            rst = cst.tile([128, 40], f32)
            nc.vector.memset(rst[:], 0.0)
            cstage = cst.tile([1, 12 * CT], f32)

            zts = [ztp.tile([128, 2, N], f8, tag=f"zt{c}", name=f"zt{c}")
                   for c in range(2)]
            accs = [accp.tile([128, N], bf16, tag=f"acc{m}", name=f"acc{m}")
                    for m in range(4)]

            # stream z8 in compute order: local col 2 first (B rows + B diag),
            # then A's cols; consts slot in behind the early chunks
            for i, (lo, hi) in enumerate(
                    ((2, 3), (0, 2), (3, 6), (6, 9), (9, 13), (13, 16))):
                for c in range(2):
                    nc.sync.dma_start(
                        zts[c][:, :, lo * CT:hi * CT],
                        z8t[c, :, :, lo * CT:hi * CT])
                if i == 1:
                    nc.sync.dma_start(dmt[:], dmask[:])
                elif i == 2:
                    nc.sync.dma_start(onest[:], ones[:])

            def sim_batch(s, m, lo, w, out_ap, slot):
                """matmul cols [lo, lo+w) vs strip s subtile m; exp -> out_ap,
                row-sum -> rs slot."""
                # all sim batches share one double-buffered tag (6 banks);
                # ones-chains use their own pool so they never block these
                row0 = (0 if s == 0 else LB) * CT + m * 128
                ps = psim.tile([128, w * CT], f32, tag="psim",
                               padded_shape=[128, 3 * CT])
                for t in range(w):
                    cl = (lo + t) * CT
                    for c in range(2):
                        nc.tensor.matmul(
                            ps[:, t * CT:(t + 1) * CT],
                            zts[c][:, :, row0:row0 + 128],
                            zts[c][:, :, cl:cl + CT],
                            start=(c == 0), stop=(c == 1), perf_mode=DR)
                nc.scalar.activation(out_ap, ps[:], AF.Exp,
                                     bias=bias10[:], scale=1.0 / TEMP,
                                     accum_out=rst[:, slot:slot + 1])

            def diag_extract(s, m, src_ap):
                tmp = scp.tile([128, CT], bf16, tag="dtmp")
                nc.vector.tensor_mul(tmp[:], src_ap,
                                     dmt[:, m * CT:(m + 1) * CT])
                nc.vector.reduce_sum(rst[:, 32 + s * 4 + m:33 + s * 4 + m],
                                     tmp[:], axis=AX.X)

            def ones_reduce(js):
                for j in js:
                    po = pone.tile([1, CT], f32, tag="pone")
                    for m in range(4):
                        nc.tensor.matmul(po[:], onest[:, 0:1],
                                         accs[m][:, j * CT:(j + 1) * CT],
                                         start=(m == 0), stop=(m == 3))
                    # DMA cannot read PSUM; bounce through SBUF on DVE
                    nc.vector.tensor_copy(
                        cstage[:, (j - 1) * CT:j * CT], po[:])
                # ship this group immediately so the final DMA isn't one
                # big serialized 30KB tail transfer
                lo, hi = (js[0] - 1) * CT, js[-1] * CT
                nc.sync.dma_start(cs[:, lo:hi], cstage[:, lo:hi])

            # strip B diag m0/m1 first: needs only local col 2 (first DMA
            # chunk) -> small batches warm up ACT while A's cols stream in;
            # m2/m3 run at the very end so the ones-chain/copy/DMA tail
            # overlaps real ACT work
            for m in range(2):
                sct = scp.tile([128, CT], bf16, tag="bdiag")
                sim_batch(1, m, LB, 1, sct[:], (4 + m) * 4 + 0)
                diag_extract(1, m, sct[:])
            # strip A: rows at local tile 0; b0 [0..2] diag+grp, b1, b2
            for m in range(4):
                sim_batch(0, m, 0, 3, accs[m][:, 0:3 * CT], (0 + m) * 4 + 0)
                diag_extract(0, m, accs[m][:, 0:CT])
            for m in range(4):
                sim_batch(0, m, 3, 3, accs[m][:, 3 * CT:6 * CT], m * 4 + 1)
            ones_reduce((1, 2))
            for m in range(4):
                sim_batch(0, m, 6, 3, accs[m][:, 6 * CT:9 * CT], m * 4 + 2)
            ones_reduce((3, 4, 5))
            # strip B: grp col local 9 (m0/m1 here, m2/m3 last so the tail
            # exps need no diag extraction), diffs [10..12], [13..15]
            for m in (0, 1):
                sim_batch(1, m, 9, 1, accs[m][:, 9 * CT:10 * CT],
                          (4 + m) * 4 + 1)
                nc.sync.dma_start(aend[m, :, 3 * CT:],
                                  accs[m][:, 9 * CT:10 * CT])
            ones_reduce((6, 7, 8))
            for m in range(4):
                sim_batch(1, m, 10, 3, accs[m][:, 10 * CT:13 * CT],
                          (4 + m) * 4 + 2)
            for m in range(4):
                sim_batch(1, m, 13, 3, accs[m][:, 13 * CT:16 * CT],
                          (4 + m) * 4 + 3)
                nc.sync.dma_start(aend[m, :, 0:3 * CT],
                                  accs[m][:, 13 * CT:16 * CT])
            for m in (2, 3):
                sct = scp.tile([128, CT], bf16, tag="bdiag")
                sim_batch(1, m, LB, 1, sct[:], (4 + m) * 4 + 0)
                diag_extract(1, m, sct[:])
            for m in (2, 3):
                sim_batch(1, m, 9, 1, accs[m][:, 9 * CT:10 * CT],
                          (4 + m) * 4 + 1)
                nc.sync.dma_start(aend[m, :, 3 * CT:],
                                  accs[m][:, 9 * CT:10 * CT])
            ones_reduce((10, 11, 12))

            nc.sync.dma_start(rs[:], rst[:])

    nc.finalize()
    return nc


def _get_built():
    global _built
    if _built is None:
        _built = _build()
    return _built


def _host_prep(embeddings, survival_times):
    E = np.asarray(embeddings, dtype=np.float32)
    t = np.asarray(survival_times, dtype=np.float32)
    q = np.quantile(t.astype(np.float64), [0.25, 0.5, 0.75])
    rg = (t[:, None].astype(np.float64) >= q[None, :]).sum(axis=1)
    counts = np.bincount(rg, minlength=G)
    assert (counts == N // G).all(), counts
    sorted_idx = np.argsort(rg, kind="stable")
    # interleaved layout: tile t holds chunk t//4 of group t%4
    tile_rows = np.concatenate(
        [sorted_idx[(tt % 4) * (N // G) + (tt // 4) * CT:]
         [:CT] for tt in range(NT)])
    norm = np.sqrt((E.astype(np.float64) ** 2).sum(axis=1))
    z = (E / np.maximum(norm, 1e-12)[:, None]).astype(np.float32)
    z = z[tile_rows]                          # [N, D] sorted-tile order
    z8 = z.astype(ml_dtypes.float8_e4m3)
    z8T = np.ascontiguousarray(z8.T)          # [D, N]

    dmask = np.zeros((128, 4 * CT), dtype=ml_dtypes.bfloat16)
    for m in range(4):
        for p in range(128):
            dmask[p, m * CT + m * 128 + p] = 1.0
    ones = np.ones((128, 16), dtype=ml_dtypes.bfloat16)

    in_maps = []
    for k in range(NCORES):
        cols = np.concatenate(
            [np.arange(CT) + ((k + PI[j]) % NT) * CT for j in range(NT)])
        zk = z8T[:, cols]                     # [512, 8192]
        z8t = np.ascontiguousarray(
            zk.reshape(2, 2, 128, N).transpose(0, 2, 1, 3))
        in_maps.append({"z8t": z8t, "dmask": dmask, "ones": ones})
    return in_maps, tile_rows


def kernel(embeddings, survival_times, censor):
    from concourse.bass_utils import run_bass_kernel_spmd

    nc = _get_built()
    in_maps, tile_rows = _host_prep(embeddings, survival_times)
    res = run_bass_kernel_spmd(nc, in_maps, list(range(NCORES)))

    S_all = np.zeros(N, dtype=np.float64)   # sorted-tile row coordinates
    S_grp = np.zeros(N, dtype=np.float64)
    dvec = np.zeros(N, dtype=np.float64)
    for k in range(NCORES):
        rsk = res.results[k]["rs"].astype(np.float64)   # [128, 40]
        cs12 = res.results[k]["cs"].astype(np.float64).reshape(12, CT)
        ae = res.results[k]["aend"].astype(np.float64).sum(axis=(0, 1))
        csk = np.empty((15, CT))          # row j-1 <-> local col j
        csk[0:8] = cs12[0:8]              # j = 1..8 (cs row 8 is unused)
        csk[8] = ae[3 * CT:]              # j = 9 via host sum
        csk[9:12] = cs12[9:12]            # j = 10..12
        csk[12:15] = ae[0:3 * CT].reshape(3, CT)   # j = 13..15
        for s, tile_t in ((0, k), (1, (k + 8) % NT)):
            base = tile_t * CT
            for m in range(4):
                rows = slice(base + m * 128, base + (m + 1) * 128)
                slots = rsk[:, (s * 4 + m) * 4:(s * 4 + m) * 4 + 4]
                if s == 0:
                    S_all[rows] += slots[:, 0] + slots[:, 1] + slots[:, 2]
                    S_grp[rows] += slots[:, 0]
                else:
                    S_all[rows] += slots.sum(axis=1)
                    S_grp[rows] += slots[:, 0] + slots[:, 1]
                dvec[rows] += rsk[:, 32 + s * 4 + m]
        for j in range(1, NT):
            tj = (k + PI[j]) % NT
            S_all[tj * CT:(tj + 1) * CT] += csk[j - 1]
            if j in (1, 2, 9):   # same-group cols (d = 4, 8, 12)
                S_grp[tj * CT:(tj + 1) * CT] += csk[j - 1]

    den = S_all - dvec
    pos = S_grp - dvec
    loss = float(np.mean(np.log(den) - np.log(pos)))
    return np.float32(loss)


# revision 32
# speedup vs baseline: 1.0261x; 1.0261x over previous
"""Distributed Trainium2 kernel for nn_ContrastiveLoss (survival contrastive loss).

Strategy (8 NeuronCores, symmetric-triangle decomposition):
  host: quantile-bin rows into 4 risk groups; lay rows out so column-tile t
        (512 rows) holds group t%4 (interleaved) -> "same group" == tile
        distance d % 4 == 0; normalize in f32 and quantize to fp8e4 (2e-2
        tolerance; measured end-to-end rel err ~1e-5).
        Each unordered tile-pair {r, c} of the 16x16 grid is computed exactly
        once: core k owns strips k (col offsets d=0..8) and k+8 (d=8..15,
        mod 16).  A fixed local permutation pi makes the per-core program
        identical (SPMD): local col j <-> global tile (k + pi[j]) % 16, with
        pi placing strip-A's same-group cols (d=4,8) right after its diagonal.
  device (per core): fp8 DoubleRow matmuls (K=512 as 2 plane-pairs, 0.5
        cycles/row) -> sim row-blocks in PSUM; one exp per <=1536-wide
        group-pure batch on ACT (scale=10, bias=-10) with accum_out giving
        masked row sums; bf16 exp tiles accumulate per row-subtile; ones-
        matmul chains give per-column sums (the transposed tiles' row sums);
        diagonal extracted via dmask on DVE.
  host: route row/col partial sums + diagonals into den/pos per row,
        loss = mean(log(den) - log(pos)).
"""
import sys

sys.path.insert(0, "/opt/trn_rl_repo")
import numpy as np
import ml_dtypes

N, D, G, NCORES = 8192, 512, 4, 8
TEMP = 0.1
CT = 512                  # col tile (rows per tile)
NT = N // CT              # 16 col tiles
PI = [0, 4, 8, 12, 1, 2, 3, 5, 6, 7, 9, 10, 11, 13, 14, 15]
# local col j of core k <-> global tile (k + PI[j]) % 16
# strip A rows = local tile 0 (d=0), same-group cols at locals {0,1,2},
# diffs at locals 4..9; strip B rows = local tile 2 (d=8) with diag (d8)
# and grp col (d12) adjacent at locals [2..3], diffs at locals 10..15
LB = 2

_built = None


def _build():
    from concourse import bacc, tile, mybir

    nc = bacc.Bacc(None, target_bir_lowering=False)
    f32 = mybir.dt.float32
    bf16 = mybir.dt.bfloat16
    f8 = mybir.dt.float8e4
    AF = mybir.ActivationFunctionType
    AX = mybir.AxisListType
    DR = mybir.MatmulPerfMode.DoubleRow

    z8t = nc.dram_tensor("z8t", [2, 128, 2, N], f8, kind="ExternalInput")
    dmask = nc.dram_tensor("dmask", [128, 4 * CT], bf16, kind="ExternalInput")
    ones = nc.dram_tensor("ones", [128, 16], bf16, kind="ExternalInput")
    rs = nc.dram_tensor("rs", [128, 40], f32, kind="ExternalOutput")
    cs = nc.dram_tensor("cs", [1, 12 * CT], f32, kind="ExternalOutput")
    # cols 13..15 and col 9 ship as raw bf16 exp tiles (summed on host):
    # their ones-chains would otherwise serialize after the final exps
    aend = nc.dram_tensor("aend", [4, 128, 4 * CT], bf16,
                          kind="ExternalOutput")

    with tile.TileContext(nc) as tc:
        with tc.tile_pool(name="cst", bufs=1) as cst, \
             tc.tile_pool(name="zt", bufs=1) as ztp, \
             tc.tile_pool(name="acc", bufs=1) as accp, \
             tc.tile_pool(name="sc", bufs=2) as scp, \
             tc.tile_pool(name="pb", bufs=2, space="PSUM") as psim, \
             tc.tile_pool(name="ps", bufs=2, space="PSUM") as pone:

            nc.scalar.add_instruction(
                mybir.InstLoadActFuncSet(
                    name=nc.get_next_instruction_name(),
                    act_func_set_id=6, ins=[], outs=[]))

            dmt = cst.tile([128, 4 * CT], bf16)
            onest = cst.tile([128, 16], bf16)
            bias10 = cst.tile([128, 1], f32)
            nc.vector.memset(bias10[:], -10.0)
            rst = cst.tile([128, 40], f32)
            nc.vector.memset(rst[:], 0.0)
            cstage = cst.tile([1, 12 * CT], f32)

            zts = [ztp.tile([128, 2, N], f8, tag=f"zt{c}", name=f"zt{c}")
                   for c in range(2)]
            accs = [accp.tile([128, N], bf16, tag=f"acc{m}", name=f"acc{m}")
                    for m in range(4)]

            # stream z8 in compute order: locals [2..3] first (B rows, B diag
            # + grp cols), then A's; consts slot in behind the early chunks
            for i, (lo, hi) in enumerate(
                    ((2, 4), (0, 2), (4, 7), (7, 10), (10, 13), (13, 16))):
                for c in range(2):
                    nc.sync.dma_start(
                        zts[c][:, :, lo * CT:hi * CT],
                        z8t[c, :, :, lo * CT:hi * CT])
                if i == 1:
                    nc.sync.dma_start(dmt[:], dmask[:])
                elif i == 2:
                    nc.sync.dma_start(onest[:], ones[:])

            def sim_batch(s, m, lo, w, out_ap, slot):
                """matmul cols [lo, lo+w) vs strip s subtile m; exp -> out_ap,
                row-sum -> rs slot."""
                # all sim batches share one double-buffered tag (6 banks);
                # ones-chains use their own pool so they never block these
                row0 = (0 if s == 0 else LB) * CT + m * 128
                ps = psim.tile([128, w * CT], f32, tag="psim",
                               padded_shape=[128, 3 * CT],
                               name=f"psim_{s}_{m}_{lo}")
                for t in range(w):
                    cl = (lo + t) * CT
                    for c in range(2):
                        nc.tensor.matmul(
                            ps[:, t * CT:(t + 1) * CT],
                            zts[c][:, :, row0:row0 + 128],
                            zts[c][:, :, cl:cl + CT],
                            start=(c == 0), stop=(c == 1), perf_mode=DR)
                nc.scalar.activation(out_ap, ps[:], AF.Exp,
                                     bias=bias10[:], scale=1.0 / TEMP,
                                     accum_out=rst[:, slot:slot + 1])

            def diag_extract(s, m, src_ap):
                tmp = scp.tile([128, CT], bf16, tag="dtmp")
                nc.vector.tensor_mul(tmp[:], src_ap,
                                     dmt[:, m * CT:(m + 1) * CT])
                nc.vector.reduce_sum(rst[:, 32 + s * 4 + m:33 + s * 4 + m],
                                     tmp[:], axis=AX.X)

            def ones_reduce(js):
                for j in js:
                    po = pone.tile([1, CT], f32, tag="pone")
                    for m in range(4):
                        nc.tensor.matmul(po[:], onest[:, 0:1],
                                         accs[m][:, j * CT:(j + 1) * CT],
                                         start=(m == 0), stop=(m == 3))
                    # DMA cannot read PSUM; bounce through SBUF on DVE
                    nc.vector.tensor_copy(
                        cstage[:, (j - 1) * CT:j * CT], po[:])
                # ship this group immediately so the final DMA isn't one
                # big serialized 30KB tail transfer
                lo, hi = (js[0] - 1) * CT, js[-1] * CT
                nc.sync.dma_start(cs[:, lo:hi], cstage[:, lo:hi])

            def b_diag_batch(m):
                # B's diag (local 2) + grp col (local 3) as one 1024 batch
                # -> scratch; diag extracted from the left half, grp col
                # shipped raw for host-side column sums
                sct = scp.tile([128, 2 * CT], bf16, tag="bdiag")
                sim_batch(1, m, LB, 2, sct[:], (4 + m) * 4 + 0)
                diag_extract(1, m, sct[:, 0:CT])
                nc.sync.dma_start(aend[m, :, 3 * CT:], sct[:, CT:2 * CT])

            # B diag m0/m1 first: needs only the first DMA chunk and warms
            # ACT while A's cols stream in; m2/m3 run at the very end so the
            # ones-chain/copy/DMA tail overlaps real ACT work
            for m in (0, 1):
                b_diag_batch(m)
            # strip A: rows at local tile 0; b0 [0..2] diag+grp, b1, b2
            for m in range(4):
                sim_batch(0, m, 0, 3, accs[m][:, 0:3 * CT], (0 + m) * 4 + 0)
                diag_extract(0, m, accs[m][:, 0:CT])
            for m in range(4):
                sim_batch(0, m, 4, 3, accs[m][:, 4 * CT:7 * CT], m * 4 + 1)
            ones_reduce((1, 2))
            for m in range(4):
                sim_batch(0, m, 7, 3, accs[m][:, 7 * CT:10 * CT], m * 4 + 2)
            ones_reduce((4, 5, 6))
            # strip B diffs [10..12], [13..15]
            for m in range(4):
                sim_batch(1, m, 10, 3, accs[m][:, 10 * CT:13 * CT],
                          (4 + m) * 4 + 1)
            ones_reduce((7, 8, 9))
            for m in range(4):
                sim_batch(1, m, 13, 3, accs[m][:, 13 * CT:16 * CT],
                          (4 + m) * 4 + 2)
                nc.sync.dma_start(aend[m, :, 0:3 * CT],
                                  accs[m][:, 13 * CT:16 * CT])
            for m in (2, 3):
                b_diag_batch(m)
            ones_reduce((10, 11, 12))

            nc.sync.dma_start(rs[:], rst[:])

    nc.finalize()
    return nc


def _get_built():
    global _built
    if _built is None:
        _built = _build()
    return _built


def _host_prep(embeddings, survival_times):
    E = np.asarray(embeddings, dtype=np.float32)
    t = np.asarray(survival_times, dtype=np.float32)
    q = np.quantile(t.astype(np.float64), [0.25, 0.5, 0.75])
    rg = (t[:, None].astype(np.float64) >= q[None, :]).sum(axis=1)
    counts = np.bincount(rg, minlength=G)
    assert (counts == N // G).all(), counts
    sorted_idx = np.argsort(rg, kind="stable")
    # interleaved layout: tile t holds chunk t//4 of group t%4
    tile_rows = np.concatenate(
        [sorted_idx[(tt % 4) * (N // G) + (tt // 4) * CT:]
         [:CT] for tt in range(NT)])
    norm = np.sqrt((E.astype(np.float64) ** 2).sum(axis=1))
    z = (E / np.maximum(norm, 1e-12)[:, None]).astype(np.float32)
    z = z[tile_rows]                          # [N, D] sorted-tile order
    z8 = z.astype(ml_dtypes.float8_e4m3)
    z8T = np.ascontiguousarray(z8.T)          # [D, N]

    dmask = np.zeros((128, 4 * CT), dtype=ml_dtypes.bfloat16)
    for m in range(4):
        for p in range(128):
            dmask[p, m * CT + m * 128 + p] = 1.0
    ones = np.ones((128, 16), dtype=ml_dtypes.bfloat16)

    in_maps = []
    for k in range(NCORES):
        cols = np.concatenate(
            [np.arange(CT) + ((k + PI[j]) % NT) * CT for j in range(NT)])
        zk = z8T[:, cols]                     # [512, 8192]
        z8t = np.ascontiguousarray(
            zk.reshape(2, 2, 128, N).transpose(0, 2, 1, 3))
        in_maps.append({"z8t": z8t, "dmask": dmask, "ones": ones})
    return in_maps, tile_rows


def kernel(embeddings, survival_times, censor):
    from concourse.bass_utils import run_bass_kernel_spmd

    nc = _get_built()
    in_maps, tile_rows = _host_prep(embeddings, survival_times)
    res = run_bass_kernel_spmd(nc, in_maps, list(range(NCORES)))

    S_all = np.zeros(N, dtype=np.float64)   # sorted-tile row coordinates
    S_grp = np.zeros(N, dtype=np.float64)
    dvec = np.zeros(N, dtype=np.float64)
    for k in range(NCORES):
        rsk = res.results[k]["rs"].astype(np.float64)   # [128, 40]
        cs12 = res.results[k]["cs"].astype(np.float64).reshape(12, CT)
        ae = res.results[k]["aend"].astype(np.float64).sum(axis=(0, 1))
        csk = np.empty((15, CT))          # row j-1 <-> local col j
        csk[0:2] = cs12[0:2]              # j = 1..2 (cs row 2 is unused)
        csk[2] = ae[3 * CT:]              # j = 3 (B grp col) via host sum
        csk[3:12] = cs12[3:12]            # j = 4..12
        csk[12:15] = ae[0:3 * CT].reshape(3, CT)   # j = 13..15
        for s, tile_t in ((0, k), (1, (k + 8) % NT)):
            base = tile_t * CT
            for m in range(4):
                rows = slice(base + m * 128, base + (m + 1) * 128)
                slots = rsk[:, (s * 4 + m) * 4:(s * 4 + m) * 4 + 4]
                S_all[rows] += slots[:, 0] + slots[:, 1] + slots[:, 2]
                S_grp[rows] += slots[:, 0]
                dvec[rows] += rsk[:, 32 + s * 4 + m]
        for j in range(1, NT):
            tj = (k + PI[j]) % NT
            S_all[tj * CT:(tj + 1) * CT] += csk[j - 1]
            if j in (1, 2, 3):   # same-group cols (d = 4, 8, 12)
                S_grp[tj * CT:(tj + 1) * CT] += csk[j - 1]

    den = S_all - dvec
    pos = S_grp - dvec
    loss = float(np.mean(np.log(den) - np.log(pos)))
    return np.float32(loss)
